# revision 9
# baseline (speedup 1.0000x reference)
"""Trainium2 Bass kernel for nn_BitwiseMLP: 3x (Linear + training-mode BatchNorm).

Math: reference computes, per layer,  h = gamma * (y - mean_B(y)) * rsqrt(var_B(y) + eps) + beta
with y = x @ W.T + b.  BatchNorm is invariant to per-feature constant shifts of y, so
  - every linear bias b_l cancels exactly,
  - the additive part of each BN affine (beta_l - a_l*mean_l) feeds the next linear as a
    per-feature constant -> also cancels under the next BN.
Only the multiplicative scales a_l = gamma_l * rsqrt(var_l + eps) propagate (folded into the
next layer's input activations), plus one final affine a2*u2 + (beta2 - a2*mean2) on the output.

Device layout: everything transposed -> activations are [features, batch_rows] so BN stats are
free-axis reductions and scales are per-partition multiplies. Batch is sharded 8 ways
(2048 rows/core); weights replicated. Matmuls in bf16 (fp32 PSUM accumulate), stats fp32,
cross-core stats via one small AllReduce per layer.
"""

import numpy as np
import ml_dtypes

# ---- problem constants (full size; hardcoded per harness contract) ----
N_CORES = 8
B_FULL = 16384
D_IN = 1024
D_H = 2048
D_OUT = 1024
BN_EPS = 1e-5

_PROG_CACHE = {}
LAST_RESULTS = None  # BassKernelResults of the most recent run (for test harness)


def build_program(R, B_total):
    """Build the per-core Bass program. R = batch rows per core (multiple of 512)."""
    import concourse.bacc as bacc
    import concourse.mybir as mybir
    import concourse.tile as tile

    f32 = mybir.dt.float32
    bf16 = mybir.dt.bfloat16
    Alu = mybir.AluOpType
    Act = mybir.ActivationFunctionType

    NT = R // 512  # n-chunks of 512 rows
    assert R % 512 == 0
    KT = [D_IN // 128, D_H // 128, D_H // 128]  # k-tiles per layer
    MT = [D_H // 128, D_H // 128, D_OUT // 128]  # m-strips per layer
    inv_B = 1.0 / float(B_total)
    GROUP = [list(range(N_CORES))]

    nc = bacc.Bacc(None, num_devices=N_CORES)

    xt_d = nc.dram_tensor("xt", [D_IN, R], bf16, kind="ExternalInput")
    w0_d = nc.dram_tensor("w0t", [D_IN, D_H], bf16, kind="ExternalInput")
    # w1/w2 pre-tiled on host: [m_strip, partition(k%128), k//128 * 128 + f]
    # so each strip DMA is one [128, KT*128] transfer with 4KB contiguous lines.
    w1_d = nc.dram_tensor("w1t", [MT[1], 128, KT[1] * 128], bf16, kind="ExternalInput")
    w2_d = nc.dram_tensor("w2t", [MT[2], 128, KT[2] * 128], bf16, kind="ExternalInput")
    g0_d = nc.dram_tensor("g0", [D_H], f32, kind="ExternalInput")
    g1_d = nc.dram_tensor("g1", [D_H], f32, kind="ExternalInput")
    g2_d = nc.dram_tensor("g2", [D_OUT], f32, kind="ExternalInput")
    b2_d = nc.dram_tensor("beta2", [D_OUT], f32, kind="ExternalInput")
    out_d = nc.dram_tensor("out", [D_OUT, R], f32, kind="ExternalOutput")

    cc_in = [nc.dram_tensor(f"cc_in{l}", [128, 2 * MT[l]], f32) for l in range(3)]
    cc_out = [nc.dram_tensor(f"cc_out{l}", [128, 2 * MT[l]], f32) for l in range(3)]

    with tile.TileContext(nc) as tc:
        import contextlib

        with contextlib.ExitStack() as ctx:
            big = ctx.enter_context(tc.tile_pool(name="big", bufs=4))
            wpool = ctx.enter_context(tc.tile_pool(name="wstrip", bufs=3))
            pspool = ctx.enter_context(tc.tile_pool(name="psum", bufs=8, space="PSUM"))
            small = ctx.enter_context(tc.tile_pool(name="small", bufs=1))
            scratch = ctx.enter_context(tc.tile_pool(name="scratch", bufs=2))

            # ---- constants / per-feature params ----
            eps_t = small.tile([128, 1], f32, tag="eps")
            nc.vector.memset(eps_t, BN_EPS)
            g_t = []
            for l, gd in enumerate((g0_d, g1_d, g2_d)):
                t = small.tile([128, MT[l]], f32, tag=f"g{l}")
                nc.sync.dma_start(out=t, in_=gd[:].rearrange("(m p) -> p m", p=128))
                g_t.append(t)
            b2_t = small.tile([128, MT[2]], f32, tag="b2")
            nc.sync.dma_start(out=b2_t, in_=b2_d[:].rearrange("(m p) -> p m", p=128))

            # ---- resident loads: xT and W0T, split fine so the first
            # (m=0, n=0) matmuls can start after ~1/4 of the data lands ----
            xt = big.tile([128, KT[0], R], bf16, tag="big")
            w0 = big.tile([128, KT[0], D_H], bf16, tag="big")
            xt_r = xt_d[:].rearrange("(j p) r -> p j r", p=128)
            w0_r = w0_d[:].rearrange("(j p) f -> p j f", p=128)
            W0CH = 4  # w0 m-chunks
            for c in range(max(NT, W0CH)):
                if c < W0CH:
                    mq = slice(c * (D_H // W0CH), (c + 1) * (D_H // W0CH))
                    for j in range(KT[0]):
                        nc.sync.dma_start(out=w0[:, j, mq], in_=w0_r[:, j, mq])
                if c < NT:
                    nq = slice(c * 512, (c + 1) * 512)
                    for j in range(KT[0]):
                        nc.sync.dma_start(out=xt[:, j, nq], in_=xt_r[:, j, nq])

            def u_pair(pool_tag, halves, dtype, strips_per_half):
                return [
                    big.tile(
                        [128, strips_per_half, R],
                        dtype,
                        tag="big",
                        name=f"{pool_tag}_{h}",
                    )
                    for h in range(halves)
                ]

            def u_slice(pair, strips_per_half, j, n=None):
                t = pair[j // strips_per_half]
                jj = j % strips_per_half
                if n is None:
                    return t[:, jj, :]
                return t[:, jj, n * 512 : (n + 1) * 512]

            def stats_block(l, BN, want_c, beta_t):
                """bn_stats partials -> per-core S/Q -> allreduce -> a [, c]."""
                mt = MT[l]
                mv = small.tile([128, mt, 2], f32, tag=f"mv{l}")
                for m in range(mt):
                    nc.vector.bn_aggr(
                        out=mv[:, m, :],
                        in_=BN[:, m * NT * 6 : (m + 1) * NT * 6],
                    )
                # S = mean*R ; Q = (var + mean^2)*R  (exact cross-core sums)
                sf = small.tile([128, 2, mt], f32, tag=f"sf{l}")
                nc.vector.tensor_scalar_mul(sf[:, 0, :], mv[:, :, 0], float(R))
                nc.vector.tensor_mul(sf[:, 1, :], mv[:, :, 0], mv[:, :, 0])
                nc.vector.tensor_add(sf[:, 1, :], sf[:, 1, :], mv[:, :, 1])
                nc.vector.tensor_scalar_mul(sf[:, 1, :], sf[:, 1, :], float(R))
                nc.sync.dma_start(out=cc_in[l][:], in_=sf)
                nc.gpsimd.collective_compute(
                    "AllReduce",
                    Alu.add,
                    replica_groups=GROUP,
                    ins=[cc_in[l][:]],
                    outs=[cc_out[l][:]],
                )
                sg = small.tile([128, 2, mt], f32, tag=f"sg{l}")
                nc.sync.dma_start(
                    out=sg, in_=cc_out[l][:].rearrange("p (s m) -> p s m", s=2)
                )
                mean = small.tile([128, mt], f32, tag=f"mean{l}")
                var = small.tile([128, mt], f32, tag=f"var{l}")
                tmp = small.tile([128, mt], f32, tag=f"tmp{l}")
                nc.vector.tensor_scalar_mul(mean, sg[:, 0, :], inv_B)
                nc.vector.tensor_scalar_mul(var, sg[:, 1, :], inv_B)
                nc.vector.tensor_mul(tmp, mean, mean)
                nc.vector.tensor_sub(var, var, tmp)
                # var <- sqrt(var + eps), then reciprocal -> rstd
                nc.scalar.activation(out=var, in_=var, func=Act.Sqrt, bias=eps_t[:, 0:1])
                nc.vector.reciprocal(out=var, in_=var)
                a = small.tile([128, mt], f32, tag=f"a{l}")
                nc.vector.tensor_mul(a, var, g_t[l])
                if not want_c:
                    return a, None
                c = small.tile([128, mt], f32, tag=f"c{l}")
                nc.vector.tensor_mul(tmp, a, mean)
                nc.vector.tensor_sub(c, beta_t, tmp)
                return a, c

            def layer(l, lhs_getter, rhs_pair, rhs_sph, dest_pair, dest_sph, dest_dt):
                """One linear layer: dest = rhs^T-layout matmul, plus bn_stats."""
                BN = small.tile([128, MT[l] * NT * 6], f32, tag=f"BN{l}")
                for m in range(MT[l]):
                    lhs = lhs_getter(m)
                    for n in range(NT):
                        ps = pspool.tile([128, 512], f32, tag="ps")
                        for j in range(KT[l]):
                            nc.tensor.matmul(
                                ps,
                                lhs(j),
                                u_slice(rhs_pair, rhs_sph, j, n),
                                start=(j == 0),
                                stop=(j == KT[l] - 1),
                            )
                        dest = u_slice(dest_pair, dest_sph, m, n)
                        idx = m * NT + n
                        nc.scalar.activation(out=dest, in_=ps, func=Act.Copy)
                        nc.vector.bn_stats(
                            out=BN[:, idx * 6 : idx * 6 + 6], in_=ps
                        )
                return BN

            # ================= layer 0 =================
            u0 = u_pair("u0", 2, bf16, MT[0] // 2)

            def lhs0(m):
                return lambda j: w0[:, j, m * 128 : (m + 1) * 128]

            def scale_strips(u, sph, kt, a):
                """In-place u *= a per strip; alternate DVE/ACT to halve latency."""
                for j in range(kt):
                    s = u_slice(u, sph, j)
                    if j % 2 == 0:
                        nc.vector.tensor_scalar_mul(s, s, a[:, j : j + 1])
                    else:
                        nc.scalar.activation(
                            out=s, in_=s, func=Act.Copy, scale=a[:, j : j + 1]
                        )

            BN0 = layer(0, lhs0, [xt], KT[0], u0, MT[0] // 2, bf16)
            a0, _ = stats_block(0, BN0, False, None)
            scale_strips(u0, MT[0] // 2, KT[1], a0)

            # ================= layer 1 =================
            u1 = u_pair("u1", 2, bf16, MT[1] // 2)

            def lhs_strip(w_dram, l):
                def getter(m):
                    w = wpool.tile([128, KT[l] * 128], bf16, tag="w", name=f"w{l}_{m}")
                    nc.sync.dma_start(out=w, in_=w_dram[m])
                    return lambda j: w[:, j * 128 : (j + 1) * 128]

                return getter

            BN1 = layer(1, lhs_strip(w1_d, 1), u0, MT[0] // 2, u1, MT[1] // 2, bf16)
            a1, _ = stats_block(1, BN1, False, None)
            scale_strips(u1, MT[1] // 2, KT[2], a1)

            # ================= layer 2 =================
            u2 = u_pair("u2", 2, f32, MT[2] // 2)
            BN2 = layer(2, lhs_strip(w2_d, 2), u1, MT[1] // 2, u2, MT[2] // 2, f32)
            a2, c2 = stats_block(2, BN2, True, b2_t)

            # ---- final affine + writeout (alternate engines + DMA queues) ----
            for m in range(MT[2]):
                s = u_slice(u2, MT[2] // 2, m)
                if m % 2 == 0:
                    nc.vector.tensor_scalar(
                        s, s, a2[:, m : m + 1], c2[:, m : m + 1], Alu.mult, Alu.add
                    )
                else:
                    nc.scalar.activation(
                        out=s,
                        in_=s,
                        func=Act.Identity,
                        bias=c2[:, m : m + 1],
                        scale=a2[:, m : m + 1],
                    )
                eng = nc.sync if m % 2 == 0 else nc.gpsimd
                eng.dma_start(out=out_d[m * 128 : (m + 1) * 128, :], in_=s)

    nc.compile()
    return nc


def _get_program(R, B_total):
    key = (R, B_total)
    if key not in _PROG_CACHE:
        _PROG_CACHE[key] = build_program(R, B_total)
    return _PROG_CACHE[key]


def prep_inputs(x, W0, W1, W2, gamma0, gamma1, gamma2, beta2, n_cores=N_CORES):
    """Host-side: transpose, cast to bf16, shard batch columns."""
    bf = ml_dtypes.bfloat16

    def strip_tiles(W):
        # W [F, K] -> [F//128 strips, 128 partitions(k%128), (K//128)*128] bf16
        # element [m, p, j*128+f] = W[m*128+f, j*128+p]
        F, Kd = W.shape
        wt = W.T.reshape(Kd // 128, 128, F // 128, 128)  # [j, p, m, f]
        return np.ascontiguousarray(wt.transpose(2, 1, 0, 3)).reshape(
            F // 128, 128, Kd // 128 * 128
        ).astype(bf)

    xT = np.ascontiguousarray(x.T)  # [D_IN, B]
    R = x.shape[0] // n_cores
    w0t = np.ascontiguousarray(W0.T).astype(bf)
    w1t = strip_tiles(np.asarray(W1, dtype=np.float32))
    w2t = strip_tiles(np.asarray(W2, dtype=np.float32))
    g0 = np.ascontiguousarray(gamma0, dtype=np.float32)
    g1 = np.ascontiguousarray(gamma1, dtype=np.float32)
    g2 = np.ascontiguousarray(gamma2, dtype=np.float32)
    b2 = np.ascontiguousarray(beta2, dtype=np.float32)
    in_maps = []
    for c in range(n_cores):
        in_maps.append(
            {
                "xt": np.ascontiguousarray(xT[:, c * R : (c + 1) * R]).astype(bf),
                "w0t": w0t,
                "w1t": w1t,
                "w2t": w2t,
                "g0": g0,
                "g1": g1,
                "g2": g2,
                "beta2": b2,
            }
        )
    return in_maps, R


def kernel(
    x,
    W0,
    b0,
    gamma0,
    beta0,
    W1,
    b1,
    gamma1,
    beta1,
    W2,
    b2,
    gamma2,
    beta2,
):
    """Full-input entry point: shard across 8 NeuronCores, run, gather.

    b0/b1/b2/beta0/beta1 cancel exactly under training-mode BatchNorm
    (shift invariance), so they are not shipped to the device.
    """
    global LAST_RESULTS
    from concourse.bass_utils import run_bass_kernel_spmd

    x = np.asarray(x, dtype=np.float32)
    B = x.shape[0]
    in_maps, R = prep_inputs(
        x, np.asarray(W0), np.asarray(W1), np.asarray(W2),
        np.asarray(gamma0), np.asarray(gamma1), np.asarray(gamma2),
        np.asarray(beta2),
    )
    nc = _get_program(R, B)
    res = run_bass_kernel_spmd(nc, in_maps, core_ids=list(range(N_CORES)))
    LAST_RESULTS = res
    out = np.empty((B, D_OUT), dtype=np.float32)
    for c in range(N_CORES):
        out[c * R : (c + 1) * R, :] = res.results[c]["out"].T
    return out


# revision 14
# speedup vs baseline: 1.1345x; 1.1345x over previous
"""Trainium2 Bass kernel for nn_BitwiseMLP: 3x (Linear + training-mode BatchNorm).

Math: reference computes, per layer,  h = gamma * (y - mean_B(y)) * rsqrt(var_B(y) + eps) + beta
with y = x @ W.T + b.  BatchNorm is invariant to per-feature constant shifts of y, so
  - every linear bias b_l cancels exactly,
  - the additive part of each BN affine (beta_l - a_l*mean_l) feeds the next linear as a
    per-feature constant -> also cancels under the next BN.
Only the multiplicative scales a_l = gamma_l * rsqrt(var_l + eps) propagate (folded into the
next layer's input activations), plus one final affine a2*u2 + (beta2 - a2*mean2) on the output.

Device layout: everything transposed -> activations are [features, batch_rows] so BN stats are
free-axis reductions and scales are per-partition multiplies. Batch is sharded 8 ways
(2048 rows/core); weights replicated. Matmuls in bf16 (fp32 PSUM accumulate), stats fp32,
cross-core stats via one small AllReduce per layer.
"""

import numpy as np
import ml_dtypes

# ---- problem constants (full size; hardcoded per harness contract) ----
N_CORES = 8
B_FULL = 16384
D_IN = 1024
D_H = 2048
D_OUT = 1024
BN_EPS = 1e-5

_PROG_CACHE = {}
LAST_RESULTS = None  # BassKernelResults of the most recent run (for test harness)


def build_program(R, B_total):
    """Build the per-core Bass program. R = batch rows per core (multiple of 512)."""
    import concourse.bacc as bacc
    import concourse.mybir as mybir
    import concourse.tile as tile

    f32 = mybir.dt.float32
    bf16 = mybir.dt.bfloat16
    Alu = mybir.AluOpType
    Act = mybir.ActivationFunctionType

    NT = R // 512  # n-chunks of 512 rows
    assert R % 512 == 0
    KT = [D_IN // 128, D_H // 128, D_H // 128]  # k-tiles per layer
    MT = [D_H // 128, D_H // 128, D_OUT // 128]  # m-strips per layer
    inv_B = 1.0 / float(B_total)
    GROUP = [list(range(N_CORES))]

    nc = bacc.Bacc(None, num_devices=N_CORES)

    xt_d = nc.dram_tensor("xt", [D_IN, R], bf16, kind="ExternalInput")
    w0_d = nc.dram_tensor("w0t", [D_IN, D_H], bf16, kind="ExternalInput")
    # w1/w2 pre-tiled on host: [m_strip, partition(k%128), k//128 * 128 + f]
    # so each strip DMA is one [128, KT*128] transfer with 4KB contiguous lines.
    w1_d = nc.dram_tensor("w1t", [MT[1], 128, KT[1] * 128], bf16, kind="ExternalInput")
    w2_d = nc.dram_tensor("w2t", [MT[2], 128, KT[2] * 128], bf16, kind="ExternalInput")
    g0_d = nc.dram_tensor("g0", [D_H], f32, kind="ExternalInput")
    g1_d = nc.dram_tensor("g1", [D_H], f32, kind="ExternalInput")
    g2_d = nc.dram_tensor("g2", [D_OUT], f32, kind="ExternalInput")
    b2_d = nc.dram_tensor("beta2", [D_OUT], f32, kind="ExternalInput")
    out_d = nc.dram_tensor("out", [D_OUT, R], f32, kind="ExternalOutput")

    # two collectives per layer (lo/hi feature halves): the lo half completes
    # while the layer's second half is still computing -> hidden latency
    cc_in = [
        [nc.dram_tensor(f"cc_in{l}_{h}", [128, MT[l]], f32) for h in range(2)]
        for l in range(3)
    ]
    cc_out = [
        [nc.dram_tensor(f"cc_out{l}_{h}", [128, MT[l]], f32) for h in range(2)]
        for l in range(3)
    ]

    with tile.TileContext(nc) as tc:
        import contextlib

        with contextlib.ExitStack() as ctx:
            big = ctx.enter_context(tc.tile_pool(name="big", bufs=4))
            wpool = ctx.enter_context(tc.tile_pool(name="wstrip", bufs=3))
            pspool = ctx.enter_context(tc.tile_pool(name="psum", bufs=8, space="PSUM"))
            small = ctx.enter_context(tc.tile_pool(name="small", bufs=1))
            scratch = ctx.enter_context(tc.tile_pool(name="scratch", bufs=2))

            # ---- constants / per-feature params ----
            eps_t = small.tile([128, 1], f32, tag="eps")
            nc.vector.memset(eps_t, BN_EPS)
            g_t = []
            for l, gd in enumerate((g0_d, g1_d, g2_d)):
                t = small.tile([128, MT[l]], f32, tag=f"g{l}")
                nc.sync.dma_start(out=t, in_=gd[:].rearrange("(m p) -> p m", p=128))
                g_t.append(t)
            b2_t = small.tile([128, MT[2]], f32, tag="b2")
            nc.sync.dma_start(out=b2_t, in_=b2_d[:].rearrange("(m p) -> p m", p=128))

            # ---- resident loads: xT and W0T on separate DMA queues ----
            xt = big.tile([128, KT[0], R], bf16, tag="big")
            w0 = big.tile([128, KT[0], D_H], bf16, tag="big")
            xt_r = xt_d[:].rearrange("(j p) r -> p j r", p=128)
            w0_r = w0_d[:].rearrange("(j p) f -> p j f", p=128)
            for j in range(KT[0]):
                nc.sync.dma_start(out=w0[:, j, :], in_=w0_r[:, j, :])
                nc.gpsimd.dma_start(out=xt[:, j, :], in_=xt_r[:, j, :])

            def u_pair(pool_tag, halves, dtype, strips_per_half):
                return [
                    big.tile(
                        [128, strips_per_half, R],
                        dtype,
                        tag="big",
                        name=f"{pool_tag}_{h}",
                    )
                    for h in range(halves)
                ]

            def u_slice(pair, strips_per_half, j, n=None):
                t = pair[j // strips_per_half]
                jj = j % strips_per_half
                if n is None:
                    return t[:, jj, :]
                return t[:, jj, n * 512 : (n + 1) * 512]

            def stats_half(l, BN, h, want_c, beta_t):
                """bn_stats partials (feature half h) -> S/Q -> allreduce -> a [, c]."""
                mh = MT[l] // 2
                m0 = h * mh
                mv = small.tile([128, mh, 2], f32, tag=f"mv{l}{h}", name=f"mv{l}{h}")
                for m in range(m0, m0 + mh):
                    nc.vector.bn_aggr(
                        out=mv[:, m - m0, :],
                        in_=BN[:, m * NT * 6 : (m + 1) * NT * 6],
                    )
                # S = mean*R ; Q = (var + mean^2)*R  (exact cross-core sums)
                sf = small.tile([128, 2, mh], f32, tag=f"sf{l}{h}", name=f"sf{l}{h}")
                nc.vector.tensor_scalar_mul(sf[:, 0, :], mv[:, :, 0], float(R))
                nc.vector.tensor_mul(sf[:, 1, :], mv[:, :, 0], mv[:, :, 0])
                nc.vector.tensor_add(sf[:, 1, :], sf[:, 1, :], mv[:, :, 1])
                nc.vector.tensor_scalar_mul(sf[:, 1, :], sf[:, 1, :], float(R))
                nc.sync.dma_start(out=cc_in[l][h][:], in_=sf)
                nc.gpsimd.collective_compute(
                    "AllReduce",
                    Alu.add,
                    replica_groups=GROUP,
                    ins=[cc_in[l][h][:]],
                    outs=[cc_out[l][h][:]],
                )
                sg = small.tile([128, 2, mh], f32, tag=f"sg{l}{h}", name=f"sg{l}{h}")
                nc.sync.dma_start(
                    out=sg, in_=cc_out[l][h][:].rearrange("p (s m) -> p s m", s=2)
                )
                mean = small.tile([128, mh], f32, tag=f"mean{l}{h}", name=f"mean{l}{h}")
                var = small.tile([128, mh], f32, tag=f"var{l}{h}", name=f"var{l}{h}")
                tmp = small.tile([128, mh], f32, tag=f"tmp{l}{h}", name=f"tmp{l}{h}")
                nc.vector.tensor_scalar_mul(mean, sg[:, 0, :], inv_B)
                nc.vector.tensor_scalar_mul(var, sg[:, 1, :], inv_B)
                nc.vector.tensor_mul(tmp, mean, mean)
                nc.vector.tensor_sub(var, var, tmp)
                # var <- sqrt(var + eps), then reciprocal -> rstd
                nc.scalar.activation(out=var, in_=var, func=Act.Sqrt, bias=eps_t[:, 0:1])
                nc.vector.reciprocal(out=var, in_=var)
                a = small.tile([128, mh], f32, tag=f"a{l}{h}", name=f"a{l}{h}")
                nc.vector.tensor_mul(a, var, g_t[l][:, m0 : m0 + mh])
                if not want_c:
                    return a, None
                c = small.tile([128, mh], f32, tag=f"c{l}{h}", name=f"c{l}{h}")
                nc.vector.tensor_mul(tmp, a, mean)
                nc.vector.tensor_sub(c, beta_t[:, m0 : m0 + mh], tmp)
                return a, c

            def stats_block(l, BN, want_c, beta_t):
                lo = stats_half(l, BN, 0, want_c, beta_t)
                hi = stats_half(l, BN, 1, want_c, beta_t)
                return (lo[0], hi[0]), (lo[1], hi[1])

            def layer(l, lhs_getter, rhs_pair, rhs_sph, dest_pair, dest_sph, dest_dt):
                """One linear layer: dest = rhs^T-layout matmul, plus bn_stats."""
                BN = small.tile([128, MT[l] * NT * 6], f32, tag=f"BN{l}")
                for m in range(MT[l]):
                    lhs = lhs_getter(m)
                    for n in range(NT):
                        ps = pspool.tile([128, 512], f32, tag="ps")
                        for j in range(KT[l]):
                            nc.tensor.matmul(
                                ps,
                                lhs(j),
                                u_slice(rhs_pair, rhs_sph, j, n),
                                start=(j == 0),
                                stop=(j == KT[l] - 1),
                            )
                        dest = u_slice(dest_pair, dest_sph, m, n)
                        idx = m * NT + n
                        nc.scalar.activation(out=dest, in_=ps, func=Act.Copy)
                        nc.vector.bn_stats(
                            out=BN[:, idx * 6 : idx * 6 + 6], in_=ps
                        )
                return BN

            # ================= layer 0 =================
            u0 = u_pair("u0", 2, bf16, MT[0] // 2)

            def lhs0(m):
                return lambda j: w0[:, j, m * 128 : (m + 1) * 128]

            def scale_strips(u, sph, kt, a_pair):
                """In-place u *= a per strip; mostly DVE, every 4th on ACT."""
                kh = kt // 2
                for j in range(kt):
                    s = u_slice(u, sph, j)
                    ac = a_pair[j // kh][:, j % kh : j % kh + 1]
                    if j % 4 == 3:
                        nc.scalar.activation(out=s, in_=s, func=Act.Copy, scale=ac)
                    else:
                        nc.vector.tensor_scalar_mul(s, s, ac)

            BN0 = layer(0, lhs0, [xt], KT[0], u0, MT[0] // 2, bf16)
            a0, _ = stats_block(0, BN0, False, None)
            scale_strips(u0, MT[0] // 2, KT[1], a0)

            # ================= layer 1 =================
            u1 = u_pair("u1", 2, bf16, MT[1] // 2)

            def lhs_strip(w_dram, l):
                def getter(m):
                    w = wpool.tile([128, KT[l] * 128], bf16, tag="w", name=f"w{l}_{m}")
                    nc.sync.dma_start(out=w, in_=w_dram[m])
                    return lambda j: w[:, j * 128 : (j + 1) * 128]

                return getter

            BN1 = layer(1, lhs_strip(w1_d, 1), u0, MT[0] // 2, u1, MT[1] // 2, bf16)
            a1, _ = stats_block(1, BN1, False, None)
            scale_strips(u1, MT[1] // 2, KT[2], a1)

            # ================= layer 2 =================
            u2 = u_pair("u2", 2, f32, MT[2] // 2)
            BN2 = layer(2, lhs_strip(w2_d, 2), u1, MT[1] // 2, u2, MT[2] // 2, f32)
            a2, c2 = stats_block(2, BN2, True, b2_t)

            # ---- final affine + writeout (alternate engines + DMA queues) ----
            mh2 = MT[2] // 2
            for m in range(MT[2]):
                s = u_slice(u2, mh2, m)
                am = a2[m // mh2][:, m % mh2 : m % mh2 + 1]
                cm = c2[m // mh2][:, m % mh2 : m % mh2 + 1]
                if m % 2 == 0:
                    nc.vector.tensor_scalar(s, s, am, cm, Alu.mult, Alu.add)
                else:
                    nc.scalar.activation(
                        out=s, in_=s, func=Act.Identity, bias=cm, scale=am
                    )
                eng = nc.sync if m % 2 == 0 else nc.gpsimd
                eng.dma_start(out=out_d[m * 128 : (m + 1) * 128, :], in_=s)

    nc.compile()
    return nc


def _get_program(R, B_total):
    key = (R, B_total)
    if key not in _PROG_CACHE:
        _PROG_CACHE[key] = build_program(R, B_total)
    return _PROG_CACHE[key]


def prep_inputs(x, W0, W1, W2, gamma0, gamma1, gamma2, beta2, n_cores=N_CORES):
    """Host-side: transpose, cast to bf16, shard batch columns."""
    bf = ml_dtypes.bfloat16

    def strip_tiles(W):
        # W [F, K] -> [F//128 strips, 128 partitions(k%128), (K//128)*128] bf16
        # element [m, p, j*128+f] = W[m*128+f, j*128+p]
        F, Kd = W.shape
        wt = W.T.reshape(Kd // 128, 128, F // 128, 128)  # [j, p, m, f]
        return np.ascontiguousarray(wt.transpose(2, 1, 0, 3)).reshape(
            F // 128, 128, Kd // 128 * 128
        ).astype(bf)

    xT = np.ascontiguousarray(x.T)  # [D_IN, B]
    R = x.shape[0] // n_cores
    w0t = np.ascontiguousarray(W0.T).astype(bf)
    w1t = strip_tiles(np.asarray(W1, dtype=np.float32))
    w2t = strip_tiles(np.asarray(W2, dtype=np.float32))
    g0 = np.ascontiguousarray(gamma0, dtype=np.float32)
    g1 = np.ascontiguousarray(gamma1, dtype=np.float32)
    g2 = np.ascontiguousarray(gamma2, dtype=np.float32)
    b2 = np.ascontiguousarray(beta2, dtype=np.float32)
    in_maps = []
    for c in range(n_cores):
        in_maps.append(
            {
                "xt": np.ascontiguousarray(xT[:, c * R : (c + 1) * R]).astype(bf),
                "w0t": w0t,
                "w1t": w1t,
                "w2t": w2t,
                "g0": g0,
                "g1": g1,
                "g2": g2,
                "beta2": b2,
            }
        )
    return in_maps, R


def kernel(
    x,
    W0,
    b0,
    gamma0,
    beta0,
    W1,
    b1,
    gamma1,
    beta1,
    W2,
    b2,
    gamma2,
    beta2,
):
    """Full-input entry point: shard across 8 NeuronCores, run, gather.

    b0/b1/b2/beta0/beta1 cancel exactly under training-mode BatchNorm
    (shift invariance), so they are not shipped to the device.
    """
    global LAST_RESULTS
    from concourse.bass_utils import run_bass_kernel_spmd

    x = np.asarray(x, dtype=np.float32)
    B = x.shape[0]
    in_maps, R = prep_inputs(
        x, np.asarray(W0), np.asarray(W1), np.asarray(W2),
        np.asarray(gamma0), np.asarray(gamma1), np.asarray(gamma2),
        np.asarray(beta2),
    )
    nc = _get_program(R, B)
    res = run_bass_kernel_spmd(nc, in_maps, core_ids=list(range(N_CORES)))
    LAST_RESULTS = res
    out = np.empty((B, D_OUT), dtype=np.float32)
    for c in range(N_CORES):
        out[c * R : (c + 1) * R, :] = res.results[c]["out"].T
    return out


# revision 17
# speedup vs baseline: 1.1399x; 1.0047x over previous
"""Trainium2 Bass kernel for nn_BitwiseMLP: 3x (Linear + training-mode BatchNorm).

Math: reference computes, per layer,  h = gamma * (y - mean_B(y)) * rsqrt(var_B(y) + eps) + beta
with y = x @ W.T + b.  BatchNorm is invariant to per-feature constant shifts of y, so
  - every linear bias b_l cancels exactly,
  - the additive part of each BN affine (beta_l - a_l*mean_l) feeds the next linear as a
    per-feature constant -> also cancels under the next BN.
Only the multiplicative scales a_l = gamma_l * rsqrt(var_l + eps) propagate (folded into the
next layer's input activations), plus one final affine a2*u2 + (beta2 - a2*mean2) on the output.

Device layout: everything transposed -> activations are [features, batch_rows] so BN stats are
free-axis reductions and scales are per-partition multiplies. Batch is sharded 8 ways
(2048 rows/core); weights replicated. Matmuls in bf16 (fp32 PSUM accumulate), stats fp32,
cross-core stats via one small AllReduce per layer.
"""

import numpy as np
import ml_dtypes

# ---- problem constants (full size; hardcoded per harness contract) ----
N_CORES = 8
B_FULL = 16384
D_IN = 1024
D_H = 2048
D_OUT = 1024
BN_EPS = 1e-5

_PROG_CACHE = {}
LAST_RESULTS = None  # BassKernelResults of the most recent run (for test harness)


def build_program(R, B_total):
    """Build the per-core Bass program. R = batch rows per core (multiple of 512)."""
    import concourse.bacc as bacc
    import concourse.mybir as mybir
    import concourse.tile as tile

    f32 = mybir.dt.float32
    bf16 = mybir.dt.bfloat16
    Alu = mybir.AluOpType
    Act = mybir.ActivationFunctionType

    NT = R // 512  # n-chunks of 512 rows
    assert R % 512 == 0
    KT = [D_IN // 128, D_H // 128, D_H // 128]  # k-tiles per layer
    MT = [D_H // 128, D_H // 128, D_OUT // 128]  # m-strips per layer
    inv_B = 1.0 / float(B_total)
    GROUP = [list(range(N_CORES))]

    nc = bacc.Bacc(None, num_devices=N_CORES)

    xt_d = nc.dram_tensor("xt", [D_IN, R], bf16, kind="ExternalInput")
    w0_d = nc.dram_tensor("w0t", [D_IN, D_H], bf16, kind="ExternalInput")
    # w1/w2 pre-tiled on host: [m_strip, partition(k%128), k//128 * 128 + f]
    # so each strip DMA is one [128, KT*128] transfer with 4KB contiguous lines.
    w1_d = nc.dram_tensor("w1t", [MT[1], 128, KT[1] * 128], bf16, kind="ExternalInput")
    w2_d = nc.dram_tensor("w2t", [MT[2], 128, KT[2] * 128], bf16, kind="ExternalInput")
    g0_d = nc.dram_tensor("g0", [D_H], f32, kind="ExternalInput")
    g1_d = nc.dram_tensor("g1", [D_H], f32, kind="ExternalInput")
    g2_d = nc.dram_tensor("g2", [D_OUT], f32, kind="ExternalInput")
    b2_d = nc.dram_tensor("beta2", [D_OUT], f32, kind="ExternalInput")
    out_d = nc.dram_tensor("out", [D_OUT, R], f32, kind="ExternalOutput")

    # two collectives per layer (lo/hi feature halves): the lo half completes
    # while the layer's second half is still computing -> hidden latency
    cc_in = [
        [nc.dram_tensor(f"cc_in{l}_{h}", [128, MT[l]], f32) for h in range(2)]
        for l in range(3)
    ]
    cc_out = [
        [nc.dram_tensor(f"cc_out{l}_{h}", [128, MT[l]], f32) for h in range(2)]
        for l in range(3)
    ]

    with tile.TileContext(nc) as tc:
        import contextlib

        with contextlib.ExitStack() as ctx:
            # one slot size (4KB/partition) for all activation/weight strips;
            # ring reuse: xt+w0 (16) -> u0 (16) -> u1 (reuses xt/w0) -> u2 (reuses u0)
            act = ctx.enter_context(tc.tile_pool(name="act", bufs=32))
            wpool = ctx.enter_context(tc.tile_pool(name="wstrip", bufs=3))
            pspool = ctx.enter_context(tc.tile_pool(name="psum", bufs=8, space="PSUM"))
            small = ctx.enter_context(tc.tile_pool(name="small", bufs=1))

            # ---- resident loads first (queue-alternate so j=0 lands early) ----
            xt_r = xt_d[:].rearrange("(j p) r -> p j r", p=128)
            w0_r = w0_d[:].rearrange("(j p) f -> p j f", p=128)
            xts, w0s = [], []
            for j in range(KT[0]):
                wt = act.tile([128, D_H], bf16, tag="act", name=f"w0_{j}")
                nc.sync.dma_start(out=wt, in_=w0_r[:, j, :])
                w0s.append(wt)
                xtile = act.tile([128, R], bf16, tag="act", name=f"xt_{j}")
                nc.gpsimd.dma_start(out=xtile, in_=xt_r[:, j, :])
                xts.append(xtile)

            # ---- constants / per-feature params ----
            eps_t = small.tile([128, 1], f32, tag="eps")
            nc.vector.memset(eps_t, BN_EPS)
            g_t = []
            for l, gd in enumerate((g0_d, g1_d, g2_d)):
                t = small.tile([128, MT[l]], f32, tag=f"g{l}", name=f"g{l}")
                nc.sync.dma_start(out=t, in_=gd[:].rearrange("(m p) -> p m", p=128))
                g_t.append(t)
            b2_t = small.tile([128, MT[2]], f32, tag="b2")
            nc.sync.dma_start(out=b2_t, in_=b2_d[:].rearrange("(m p) -> p m", p=128))

            def u_strips(pool_tag, count, dtype, cols):
                return [
                    act.tile([128, cols], dtype, tag="act", name=f"{pool_tag}_{j}")
                    for j in range(count)
                ]

            def stats_half(l, BN, h, want_c, beta_t):
                """bn_stats partials (feature half h) -> S/Q -> allreduce -> a [, c]."""
                mh = MT[l] // 2
                m0 = h * mh
                mv = small.tile([128, mh, 2], f32, tag=f"mv{l}{h}", name=f"mv{l}{h}")
                for m in range(m0, m0 + mh):
                    nc.vector.bn_aggr(
                        out=mv[:, m - m0, :],
                        in_=BN[:, m * NT * 6 : (m + 1) * NT * 6],
                    )
                # S = mean*R ; Q = (var + mean^2)*R  (exact cross-core sums)
                sf = small.tile([128, 2, mh], f32, tag=f"sf{l}{h}", name=f"sf{l}{h}")
                nc.vector.tensor_scalar_mul(sf[:, 0, :], mv[:, :, 0], float(R))
                nc.vector.tensor_mul(sf[:, 1, :], mv[:, :, 0], mv[:, :, 0])
                nc.vector.tensor_add(sf[:, 1, :], sf[:, 1, :], mv[:, :, 1])
                nc.vector.tensor_scalar_mul(sf[:, 1, :], sf[:, 1, :], float(R))
                nc.sync.dma_start(out=cc_in[l][h][:], in_=sf)
                nc.gpsimd.collective_compute(
                    "AllReduce",
                    Alu.add,
                    replica_groups=GROUP,
                    ins=[cc_in[l][h][:]],
                    outs=[cc_out[l][h][:]],
                )
                sg = small.tile([128, 2, mh], f32, tag=f"sg{l}{h}", name=f"sg{l}{h}")
                nc.sync.dma_start(
                    out=sg, in_=cc_out[l][h][:].rearrange("p (s m) -> p s m", s=2)
                )
                mean = small.tile([128, mh], f32, tag=f"mean{l}{h}", name=f"mean{l}{h}")
                var = small.tile([128, mh], f32, tag=f"var{l}{h}", name=f"var{l}{h}")
                tmp = small.tile([128, mh], f32, tag=f"tmp{l}{h}", name=f"tmp{l}{h}")
                nc.vector.tensor_scalar_mul(mean, sg[:, 0, :], inv_B)
                nc.vector.tensor_scalar_mul(var, sg[:, 1, :], inv_B)
                nc.vector.tensor_mul(tmp, mean, mean)
                nc.vector.tensor_sub(var, var, tmp)
                # var <- sqrt(var + eps), then reciprocal -> rstd
                nc.scalar.activation(out=var, in_=var, func=Act.Sqrt, bias=eps_t[:, 0:1])
                nc.vector.reciprocal(out=var, in_=var)
                a = small.tile([128, mh], f32, tag=f"a{l}{h}", name=f"a{l}{h}")
                nc.vector.tensor_mul(a, var, g_t[l][:, m0 : m0 + mh])
                if not want_c:
                    return a, None
                c = small.tile([128, mh], f32, tag=f"c{l}{h}", name=f"c{l}{h}")
                nc.vector.tensor_mul(tmp, a, mean)
                nc.vector.tensor_sub(c, beta_t[:, m0 : m0 + mh], tmp)
                return a, c

            def stats_block(l, BN, want_c, beta_t):
                lo = stats_half(l, BN, 0, want_c, beta_t)
                hi = stats_half(l, BN, 1, want_c, beta_t)
                return (lo[0], hi[0]), (lo[1], hi[1])

            def layer(l, lhs_getter, rhs_at, dest_at):
                """One linear layer, k-outer (weights reused across n), bn_stats."""
                BN = small.tile([128, MT[l] * NT * 6], f32, tag=f"BN{l}", name=f"BN{l}")
                for m in range(MT[l]):
                    lhs = lhs_getter(m)
                    pss = [
                        pspool.tile([128, 512], f32, tag="ps", name=f"ps{l}_{m}_{n}")
                        for n in range(NT)
                    ]
                    for j in range(KT[l]):
                        w_ap = lhs(j)
                        for n in range(NT):
                            nc.tensor.matmul(
                                pss[n],
                                w_ap,
                                rhs_at(j, n),
                                start=(j == 0),
                                stop=(j == KT[l] - 1),
                            )
                    for n in range(NT):
                        idx = m * NT + n
                        nc.scalar.activation(
                            out=dest_at(m, n), in_=pss[n], func=Act.Copy
                        )
                        nc.vector.bn_stats(
                            out=BN[:, idx * 6 : idx * 6 + 6], in_=pss[n]
                        )
                return BN

            def strips_rhs(strips):
                return lambda j, n: strips[j][:, n * 512 : (n + 1) * 512]

            def scale_strips(strips, a_pair):
                """In-place u *= a per strip; mostly DVE, every 4th on ACT."""
                kt = len(strips)
                kh = kt // 2
                for j in range(kt):
                    s = strips[j][:]
                    ac = a_pair[j // kh][:, j % kh : j % kh + 1]
                    if j % 4 == 3:
                        nc.scalar.activation(out=s, in_=s, func=Act.Copy, scale=ac)
                    else:
                        nc.vector.tensor_scalar_mul(s, s, ac)

            # ================= layer 0 =================
            u0 = u_strips("u0", MT[0], bf16, R)

            def lhs0(m):
                return lambda j: w0s[j][:, m * 128 : (m + 1) * 128]

            BN0 = layer(0, lhs0, strips_rhs(xts), lambda m, n: strips_rhs(u0)(m, n))
            a0, _ = stats_block(0, BN0, False, None)
            scale_strips(u0, a0)

            # ================= layer 1 =================
            u1 = u_strips("u1", MT[1], bf16, R)

            def lhs_strip(w_dram, l):
                def getter(m):
                    w = wpool.tile([128, KT[l] * 128], bf16, tag="w", name=f"w{l}_{m}")
                    nc.sync.dma_start(out=w, in_=w_dram[m])
                    return lambda j: w[:, j * 128 : (j + 1) * 128]

                return getter

            BN1 = layer(1, lhs_strip(w1_d, 1), strips_rhs(u0), strips_rhs(u1))
            a1, _ = stats_block(1, BN1, False, None)
            scale_strips(u1, a1)

            # ================= layer 2 =================
            # u2 fp32 strips split in column halves so slots match the 4KB ring
            NH2 = 2 if NT >= 2 else 1
            C2 = R // NH2
            CPH = NT // NH2  # 512-chunks per half
            u2 = u_strips("u2", NH2 * MT[2], f32, C2)

            def u2_at(m, n):
                return u2[NH2 * m + n // CPH][
                    :, (n % CPH) * 512 : (n % CPH) * 512 + 512
                ]

            BN2 = layer(2, lhs_strip(w2_d, 2), strips_rhs(u1), u2_at)
            a2, c2 = stats_block(2, BN2, True, b2_t)

            # ---- final affine + writeout (alternate engines + DMA queues) ----
            mh2 = MT[2] // 2
            for idx in range(NH2 * MT[2]):
                m, h = idx // NH2, idx % NH2
                s = u2[idx][:]
                am = a2[m // mh2][:, m % mh2 : m % mh2 + 1]
                cm = c2[m // mh2][:, m % mh2 : m % mh2 + 1]
                if idx % 2 == 0:
                    nc.vector.tensor_scalar(s, s, am, cm, Alu.mult, Alu.add)
                else:
                    nc.scalar.activation(
                        out=s, in_=s, func=Act.Identity, bias=cm, scale=am
                    )
                eng = nc.sync if idx % 2 == 0 else nc.gpsimd
                eng.dma_start(
                    out=out_d[m * 128 : (m + 1) * 128, h * C2 : (h + 1) * C2],
                    in_=s,
                )

    nc.compile()
    return nc


def _get_program(R, B_total):
    key = (R, B_total)
    if key not in _PROG_CACHE:
        _PROG_CACHE[key] = build_program(R, B_total)
    return _PROG_CACHE[key]


def prep_inputs(x, W0, W1, W2, gamma0, gamma1, gamma2, beta2, n_cores=N_CORES):
    """Host-side: transpose, cast to bf16, shard batch columns."""
    bf = ml_dtypes.bfloat16

    def strip_tiles(W):
        # W [F, K] -> [F//128 strips, 128 partitions(k%128), (K//128)*128] bf16
        # element [m, p, j*128+f] = W[m*128+f, j*128+p]
        F, Kd = W.shape
        wt = W.T.reshape(Kd // 128, 128, F // 128, 128)  # [j, p, m, f]
        return np.ascontiguousarray(wt.transpose(2, 1, 0, 3)).reshape(
            F // 128, 128, Kd // 128 * 128
        ).astype(bf)

    xT = np.ascontiguousarray(x.T)  # [D_IN, B]
    R = x.shape[0] // n_cores
    w0t = np.ascontiguousarray(W0.T).astype(bf)
    w1t = strip_tiles(np.asarray(W1, dtype=np.float32))
    w2t = strip_tiles(np.asarray(W2, dtype=np.float32))
    g0 = np.ascontiguousarray(gamma0, dtype=np.float32)
    g1 = np.ascontiguousarray(gamma1, dtype=np.float32)
    g2 = np.ascontiguousarray(gamma2, dtype=np.float32)
    b2 = np.ascontiguousarray(beta2, dtype=np.float32)
    in_maps = []
    for c in range(n_cores):
        in_maps.append(
            {
                "xt": np.ascontiguousarray(xT[:, c * R : (c + 1) * R]).astype(bf),
                "w0t": w0t,
                "w1t": w1t,
                "w2t": w2t,
                "g0": g0,
                "g1": g1,
                "g2": g2,
                "beta2": b2,
            }
        )
    return in_maps, R


def kernel(
    x,
    W0,
    b0,
    gamma0,
    beta0,
    W1,
    b1,
    gamma1,
    beta1,
    W2,
    b2,
    gamma2,
    beta2,
):
    """Full-input entry point: shard across 8 NeuronCores, run, gather.

    b0/b1/b2/beta0/beta1 cancel exactly under training-mode BatchNorm
    (shift invariance), so they are not shipped to the device.
    """
    global LAST_RESULTS
    from concourse.bass_utils import run_bass_kernel_spmd

    x = np.asarray(x, dtype=np.float32)
    B = x.shape[0]
    in_maps, R = prep_inputs(
        x, np.asarray(W0), np.asarray(W1), np.asarray(W2),
        np.asarray(gamma0), np.asarray(gamma1), np.asarray(gamma2),
        np.asarray(beta2),
    )
    nc = _get_program(R, B)
    res = run_bass_kernel_spmd(nc, in_maps, core_ids=list(range(N_CORES)))
    LAST_RESULTS = res
    out = np.empty((B, D_OUT), dtype=np.float32)
    for c in range(N_CORES):
        out[c * R : (c + 1) * R, :] = res.results[c]["out"].T
    return out
